# revision 6
# baseline (speedup 1.0000x reference)
"""Trainium2 Bass kernel for a FlowNet-style CorrelationLayer.

out[0, j*7+i, h, w] = sum_c x[0,c,h,w] * y[0,c,h+j-3,w+i-3]   (zero-padded y)

Shapes: x, y = [1, 128, 384, 512] fp32  ->  out = [1, 49, 384, 512] fp32.

Strategy
--------
* Shard H (rows) across the 8 NeuronCores: core k computes output rows
  [48k, 48k+48).  The y halo (3 rows each side) is sliced on the host from
  the full input, so no inter-core communication is needed.
* Per core, the C=128 contraction runs on the TensorEngine as "all-pairs"
  patch matmuls: lhsT = a 4x8 pixel patch of x (M=32 columns, K=C=128),
  rhs = the matching 10x14 halo patch of y (N=140 columns).  Entry
  (m=(a,b), n=(al,be)) of the PSUM block is the correlation of x pixel
  (a,b) with y pixel (al-3, be-3) relative to the patch origin, so the 49
  shift planes live on 49 diagonals of each block.  Four patches are packed
  into the 128 PE columns via 32-wide tile_position col-tiling so
  partitions (and hence DMA width) stay full.  The 4x8 patch minimizes the
  halo area (10*14=140) per 32 pixels, cutting the dumped-block traffic to
  140/49 = 2.86x the true output (the 8x8 variant dumps 196/49 = 4x).
* Diagonal extraction is not expressible with uniform per-partition access
  patterns on any engine (and GPSIMD gathers share indices across each
  16-partition group), so each PSUM block is cast to fp16 and dumped
  whole to DRAM; the final banded gather is a cheap numpy fancy-index on
  the host.  Inputs are also shipped as fp16 (quantization error ~4e-4
  relative, well within tolerance).  Total HBM traffic per core is
  ~20.3 MB, close to the memory roofline.
"""

import numpy as np

import concourse.bass as bass  # noqa: F401  (AP types pulled in transitively)
import concourse.tile as tile
from concourse import bacc, mybir
from concourse.bass_utils import run_bass_kernel_spmd

B, C, H, W = 1, 128, 384, 512
NCORES = 8
HB = H // NCORES          # 48 output rows per core
PA, PB = 4, 8             # x patch: 4 rows x 8 cols = 32 = M per matmul
HA, HW_ = PA + 6, PB + 6  # y halo patch: 10 x 14
NF = HA * HW_             # 140 = N (matmul free size)
PR = HB // PA             # 12 patch-rows
PW = W // PB              # 64 patch-cols
G = 4                     # patches packed per 128 partitions (col tiles)
QG = PW // G              # 16 groups of 4 patches per patch-row

F16 = mybir.dt.float16

_PROGRAM = None


def _build_program():
    nc = bacc.Bacc("TRN2", target_bir_lowering=False, debug=False)

    # x is pre-tiled on the host to [C, patch, m] so each patch's 32 weight
    # columns are contiguous (walrus requires a single free dim on the
    # stationary matmul operand).
    xb = nc.declare_dram_parameter("xb", [C, PR * PW, PA * PB], F16, isOutput=False)
    yb = nc.declare_dram_parameter("yb", [C, HB + 6, W + 6], F16, isOutput=False)
    corr = nc.declare_dram_parameter("corr", [PR, 128, QG, NF], F16, isOutput=True)

    with tile.TileContext(nc) as tc:
        with (
            tc.tile_pool(name="xpool", bufs=1) as xpool,
            tc.tile_pool(name="ypool", bufs=1) as ypool,
            tc.tile_pool(name="psum", bufs=6, space="PSUM") as psum_pool,
            tc.tile_pool(name="stage", bufs=2) as stage_pool,
        ):
            X = xpool.tile([C, PR * PW, PA * PB], F16)
            Y = ypool.tile([C, HB + 6, W + 6], F16)

            # Issue input loads in the order the patch-row pipeline consumes
            # them (the HW queue drains FIFO): patch-row pr needs X chunk pr
            # and Y rows [4pr, 4pr+10) = Y chunks pr, pr+1, pr+2.
            def load_x(pr):
                nc.sync.dma_start(
                    X[:, pr * PW : (pr + 1) * PW, :], xb[:, pr * PW : (pr + 1) * PW, :]
                )

            def load_y(ch):  # Y chunk = 4 rows (last chunk 2 rows)
                r0, r1 = ch * 4, min(ch * 4 + 4, HB + 6)
                nc.sync.dma_start(Y[:, r0:r1, :], yb[:, r0:r1, :])

            load_x(0)
            load_y(0)
            load_y(1)
            load_y(2)
            for pr in range(1, PR):
                load_x(pr)
                load_y(pr + 2)

            for pr in range(PR):
                # One staging buffer and two output DMAs per patch-row keep
                # the Sync sequencer's per-DMA dispatch (~0.6us) off the
                # critical path.
                st = stage_pool.tile([128, QG, NF], F16)
                for g in range(QG):
                    # Four 4x8 patches (32-wide col tiles) share one PSUM
                    # block; their evacuation is a single strided copy.
                    ps = psum_pool.tile([128, NF], mybir.dt.float32)
                    for t in range(G):
                        wp = g * G + t
                        lhsT = X[:, pr * PW + wp, :]
                        rhs = Y[
                            :, pr * PA : pr * PA + HA, wp * PB : wp * PB + HW_
                        ]
                        nc.tensor.matmul(
                            ps[32 * t : 32 * (t + 1), :],
                            lhsT,
                            rhs,
                            start=True,
                            stop=True,
                            tile_position=(0, 32 * t),
                        )
                    dst = st[:, g, :]
                    # Alternate evacuation between DVE and ACT so neither
                    # becomes the bottleneck.
                    if g % 2 == 0:
                        nc.vector.tensor_copy(dst, ps)
                    else:
                        nc.scalar.copy(dst, ps)
                    if g == QG // 2 - 1:
                        # First half of the row band is done — ship it while
                        # the second half is still being computed.
                        nc.sync.dma_start(
                            corr[pr, :, : QG // 2], st[:, : QG // 2, :]
                        )
                nc.sync.dma_start(corr[pr, :, QG // 2 :], st[:, QG // 2 :, :])

    nc.compile()
    return nc


def _program():
    global _PROGRAM
    if _PROGRAM is None:
        _PROGRAM = _build_program()
    return _PROGRAM


def _make_in_maps(x: np.ndarray, y: np.ndarray):
    x0 = np.asarray(x[0]).astype(np.float16)
    # [C, H, W] -> [C, H/PA, PA, PW, PB] -> [C, H/PA, PW, PA, PB]
    xt = x0.reshape(C, H // PA, PA, PW, PB).transpose(0, 1, 3, 2, 4)
    xt = np.ascontiguousarray(xt.reshape(C, H // PA * PW, PA * PB))
    yp = np.zeros((C, H + 6, W + 6), np.float16)
    yp[:, 3 : 3 + H, 3 : 3 + W] = y[0]
    in_maps = []
    for k in range(NCORES):
        in_maps.append(
            {
                "xb": np.ascontiguousarray(xt[:, k * PR * PW : (k + 1) * PR * PW, :]),
                "yb": np.ascontiguousarray(yp[:, k * HB : k * HB + HB + 6, :]),
            }
        )
    return in_maps


_GATHER_IDX = None


def _gather_indices():
    global _GATHER_IDX
    if _GATHER_IDX is None:
        a = np.arange(PA)[:, None, None, None]
        b = np.arange(PB)[None, :, None, None]
        j = np.arange(7)[None, None, :, None]
        i = np.arange(7)[None, None, None, :]
        # n offset for pixel (a, b) and shift (j, i), flattened over (j, i)
        n_idx = ((a + j) * HW_ + (b + i)).reshape(1, 1, PA, PB, 1, 49)
        _GATHER_IDX = np.ascontiguousarray(n_idx)
    return _GATHER_IDX


def _gather_core(corr_k: np.ndarray) -> np.ndarray:
    """[PR, 128, QG, NF] -> [49, HB, W] band of the output."""
    n_idx = _gather_indices()
    ck = corr_k.reshape(PR, G, PA, PB, QG, NF)
    g = np.take_along_axis(ck, n_idx, axis=5)
    # g: [PR, t, a, b, q, 49]; out[s, pr*PA+a, (q*G+t)*PB+b] = g[pr, t, a, b, q, s]
    g = g.transpose(5, 0, 2, 4, 1, 3).reshape(49, HB, W)
    return g


def _run(in_maps, trace=False, **kw):
    return run_bass_kernel_spmd(
        _program(), in_maps, core_ids=list(range(NCORES)), trace=trace, **kw
    )


def kernel(x: np.ndarray, y: np.ndarray) -> np.ndarray:
    x = np.asarray(x)
    y = np.asarray(y)
    res = _run(_make_in_maps(x, y)).results
    out = np.empty((1, 49, H, W), np.float32)
    for k in range(NCORES):
        out[0, :, k * HB : (k + 1) * HB, :] = _gather_core(
            np.asarray(res[k]["corr"])
        ).astype(np.float32)
    return out


# revision 8
# speedup vs baseline: 1.0874x; 1.0874x over previous
"""Trainium2 Bass kernel for a FlowNet-style CorrelationLayer.

out[0, j*7+i, h, w] = sum_c x[0,c,h,w] * y[0,c,h+j-3,w+i-3]   (zero-padded y)

Shapes: x, y = [1, 128, 384, 512] fp32  ->  out = [1, 49, 384, 512] fp32.

Strategy
--------
* Shard H (rows) across the 8 NeuronCores: core k computes output rows
  [48k, 48k+48).  The y halo (3 rows each side) is sliced on the host from
  the full input, so no inter-core communication is needed.
* Per core, the C=128 contraction runs on the TensorEngine as "all-pairs"
  patch matmuls: lhsT = a 4x8 pixel patch of x (M=32 columns, K=C=128),
  rhs = the matching 10x14 halo patch of y (N=140 columns).  Entry
  (m=(a,b), n=(al,be)) of the PSUM block is the correlation of x pixel
  (a,b) with y pixel (al-3, be-3) relative to the patch origin, so the 49
  shift planes live on 49 diagonals of each block.  Four patches are packed
  into the 128 PE columns via 32-wide tile_position col-tiling so
  partitions (and hence DMA width) stay full.  The 4x8 patch minimizes the
  halo area (10*14=140) per 32 pixels, cutting the dumped-block traffic to
  140/49 = 2.86x the true output (the 8x8 variant dumps 196/49 = 4x).
* Diagonal extraction is not expressible with uniform per-partition access
  patterns on any engine (and GPSIMD gathers share indices across each
  16-partition group), so each PSUM block is cast to fp16 and dumped
  whole to DRAM; the final banded gather is a cheap numpy fancy-index on
  the host.  Inputs are also shipped as fp16 (quantization error ~4e-4
  relative, well within tolerance).  Total HBM traffic per core is
  ~20.3 MB, close to the memory roofline.
"""

import numpy as np

import concourse.bass as bass  # noqa: F401  (AP types pulled in transitively)
import concourse.tile as tile
from concourse import bacc, mybir
from concourse.bass_utils import run_bass_kernel_spmd

B, C, H, W = 1, 128, 384, 512
NCORES = 8
HB = H // NCORES          # 48 output rows per core
PA, PB = 4, 8             # x patch: 4 rows x 8 cols = 32 = M per matmul
HA, HW_ = PA + 6, PB + 6  # y halo patch: 10 x 14
NF = HA * HW_             # 140 = N (matmul free size)
PR = HB // PA             # 12 patch-rows
PW = W // PB              # 64 patch-cols
G = 4                     # patches packed per 128 partitions (col tiles)
QG = PW // G              # 16 groups of 4 patches per patch-row

F16 = mybir.dt.float16

_PROGRAM = None


def _build_program():
    nc = bacc.Bacc("TRN2", target_bir_lowering=False, debug=False)

    # x is pre-tiled on the host to [C, patch, m] so each patch's 32 weight
    # columns are contiguous (walrus requires a single free dim on the
    # stationary matmul operand).
    xb = nc.declare_dram_parameter("xb", [C, PR * PW, PA * PB], F16, isOutput=False)
    yb = nc.declare_dram_parameter("yb", [C, HB + 6, W + 6], F16, isOutput=False)
    corr = nc.declare_dram_parameter("corr", [PR, 128, QG, NF], F16, isOutput=True)

    with tile.TileContext(nc) as tc:
        with (
            tc.tile_pool(name="xpool", bufs=1) as xpool,
            tc.tile_pool(name="ypool", bufs=1) as ypool,
            tc.tile_pool(name="psum", bufs=6, space="PSUM") as psum_pool,
            tc.tile_pool(name="stage", bufs=2) as stage_pool,
        ):
            X = xpool.tile([C, PR * PW, PA * PB], F16)
            Y = ypool.tile([C, HB + 6, W + 6], F16)

            # Issue input loads in the order the patch-row pipeline consumes
            # them (the HW queue drains FIFO).  Few, large transfers keep the
            # DMA ring at full bandwidth: x in 2-patch-row chunks (1.05 MB),
            # y in 8-row chunks (1.06 MB).  Patch-row pr needs X chunk pr//2
            # and Y rows [4pr, 4pr+10) = Y chunks pr//2 and pr//2+1.
            def load_x(cx):  # 2 patch-rows of x
                nc.sync.dma_start(
                    X[:, cx * 2 * PW : (cx + 1) * 2 * PW, :],
                    xb[:, cx * 2 * PW : (cx + 1) * 2 * PW, :],
                )

            def load_y(ch):  # Y chunk = 8 rows (last chunk 6 rows)
                r0, r1 = ch * 8, min(ch * 8 + 8, HB + 6)
                nc.sync.dma_start(Y[:, r0:r1, :], yb[:, r0:r1, :])

            load_x(0)
            load_y(0)
            load_y(1)
            for cx in range(1, PR // 2):
                load_x(cx)
                load_y(cx + 1)

            for pr in range(PR):
                # One staging buffer and one output DMA per patch-row keep
                # the Sync sequencer's per-DMA dispatch (~0.6us) off the
                # critical path.
                st = stage_pool.tile([128, QG, NF], F16)
                for gg in range(0, QG, 2):
                    # Eight 4x8 patches (two col-tiled quads) share one PSUM
                    # bank; their evacuation is a single strided copy.
                    ps = psum_pool.tile([128, 2, NF], mybir.dt.float32)
                    for s in range(2):
                        g = gg + s
                        for t in range(G):
                            wp = g * G + t
                            lhsT = X[:, pr * PW + wp, :]
                            rhs = Y[
                                :, pr * PA : pr * PA + HA, wp * PB : wp * PB + HW_
                            ]
                            nc.tensor.matmul(
                                ps[32 * t : 32 * (t + 1), s, :],
                                lhsT,
                                rhs,
                                start=True,
                                stop=True,
                                tile_position=(0, 32 * t),
                            )
                    dst = st[:, gg : gg + 2, :]
                    # Alternate evacuation between DVE and ACT so neither
                    # becomes the bottleneck.
                    if (gg // 2) % 2 == 0:
                        nc.vector.tensor_copy(dst, ps)
                    else:
                        nc.scalar.copy(dst, ps)
                nc.sync.dma_start(corr[pr], st)

    nc.compile()
    return nc


def _program():
    global _PROGRAM
    if _PROGRAM is None:
        _PROGRAM = _build_program()
    return _PROGRAM


def _make_in_maps(x: np.ndarray, y: np.ndarray):
    x0 = np.asarray(x[0]).astype(np.float16)
    # [C, H, W] -> [C, H/PA, PA, PW, PB] -> [C, H/PA, PW, PA, PB]
    xt = x0.reshape(C, H // PA, PA, PW, PB).transpose(0, 1, 3, 2, 4)
    xt = np.ascontiguousarray(xt.reshape(C, H // PA * PW, PA * PB))
    yp = np.zeros((C, H + 6, W + 6), np.float16)
    yp[:, 3 : 3 + H, 3 : 3 + W] = y[0]
    in_maps = []
    for k in range(NCORES):
        in_maps.append(
            {
                "xb": np.ascontiguousarray(xt[:, k * PR * PW : (k + 1) * PR * PW, :]),
                "yb": np.ascontiguousarray(yp[:, k * HB : k * HB + HB + 6, :]),
            }
        )
    return in_maps


_GATHER_IDX = None


def _gather_indices():
    global _GATHER_IDX
    if _GATHER_IDX is None:
        a = np.arange(PA)[:, None, None, None]
        b = np.arange(PB)[None, :, None, None]
        j = np.arange(7)[None, None, :, None]
        i = np.arange(7)[None, None, None, :]
        # n offset for pixel (a, b) and shift (j, i), flattened over (j, i)
        n_idx = ((a + j) * HW_ + (b + i)).reshape(1, 1, PA, PB, 1, 49)
        _GATHER_IDX = np.ascontiguousarray(n_idx)
    return _GATHER_IDX


def _gather_core(corr_k: np.ndarray) -> np.ndarray:
    """[PR, 128, QG, NF] -> [49, HB, W] band of the output."""
    n_idx = _gather_indices()
    ck = corr_k.reshape(PR, G, PA, PB, QG, NF)
    g = np.take_along_axis(ck, n_idx, axis=5)
    # g: [PR, t, a, b, q, 49]; out[s, pr*PA+a, (q*G+t)*PB+b] = g[pr, t, a, b, q, s]
    g = g.transpose(5, 0, 2, 4, 1, 3).reshape(49, HB, W)
    return g


def _run(in_maps, trace=False, **kw):
    return run_bass_kernel_spmd(
        _program(), in_maps, core_ids=list(range(NCORES)), trace=trace, **kw
    )


def kernel(x: np.ndarray, y: np.ndarray) -> np.ndarray:
    x = np.asarray(x)
    y = np.asarray(y)
    res = _run(_make_in_maps(x, y)).results
    out = np.empty((1, 49, H, W), np.float32)
    for k in range(NCORES):
        out[0, :, k * HB : (k + 1) * HB, :] = _gather_core(
            np.asarray(res[k]["corr"])
        ).astype(np.float32)
    return out


# revision 9
# speedup vs baseline: 1.1846x; 1.0894x over previous
"""Trainium2 Bass kernel for a FlowNet-style CorrelationLayer.

out[0, j*7+i, h, w] = sum_c x[0,c,h,w] * y[0,c,h+j-3,w+i-3]   (zero-padded y)

Shapes: x, y = [1, 128, 384, 512] fp32  ->  out = [1, 49, 384, 512] fp32.

Strategy
--------
* Shard H (rows) across the 8 NeuronCores: core k computes output rows
  [48k, 48k+48).  The y halo (3 rows each side) is sliced on the host from
  the full input, so no inter-core communication is needed.
* Per core, the C=128 contraction runs on the TensorEngine as "all-pairs"
  patch matmuls: lhsT = a 4x8 pixel patch of x (M=32 columns, K=C=128),
  rhs = the matching 10x14 halo patch of y (N=140 columns).  Entry
  (m=(a,b), n=(al,be)) of the PSUM block is the correlation of x pixel
  (a,b) with y pixel (al-3, be-3) relative to the patch origin, so the 49
  shift planes live on 49 diagonals of each block.  Four patches are packed
  into the 128 PE columns via 32-wide tile_position col-tiling so
  partitions (and hence DMA width) stay full.  The 4x8 patch minimizes the
  halo area (10*14=140) per 32 pixels, cutting the dumped-block traffic to
  140/49 = 2.86x the true output (the 8x8 variant dumps 196/49 = 4x).
* Diagonal extraction is not expressible with uniform per-partition access
  patterns on any engine (and GPSIMD gathers share indices across each
  16-partition group), so each PSUM block is cast to fp16 and dumped
  whole to DRAM; the final banded gather is a cheap numpy fancy-index on
  the host.  Inputs are also shipped as fp16 (quantization error ~4e-4
  relative, well within tolerance).  Total HBM traffic per core is
  ~20.3 MB, close to the memory roofline.
"""

import numpy as np

import concourse.bass as bass  # noqa: F401  (AP types pulled in transitively)
import concourse.tile as tile
from concourse import bacc, mybir
from concourse.bass_utils import run_bass_kernel_spmd

B, C, H, W = 1, 128, 384, 512
NCORES = 8
HB = H // NCORES          # 48 output rows per core
PA, PB = 4, 8             # x patch: 4 rows x 8 cols = 32 = M per matmul
HA, HW_ = PA + 6, PB + 6  # y halo patch: 10 x 14
NF = HA * HW_             # 140 = N (matmul free size)
PR = HB // PA             # 12 patch-rows
PW = W // PB              # 64 patch-cols
G = 4                     # patches packed per 128 partitions (col tiles)
QG = PW // G              # 16 groups of 4 patches per patch-row

F16 = mybir.dt.float16

_PROGRAM = None


def _build_program():
    nc = bacc.Bacc("TRN2", target_bir_lowering=False, debug=False)

    # x is pre-tiled on the host to [C, patch, m] so each patch's 32 weight
    # columns are contiguous (walrus requires a single free dim on the
    # stationary matmul operand).
    xb = nc.declare_dram_parameter("xb", [C, PR * PW, PA * PB], F16, isOutput=False)
    yb = nc.declare_dram_parameter("yb", [C, HB + 6, W + 6], F16, isOutput=False)
    corr = nc.declare_dram_parameter("corr", [PR, 128, QG, NF], F16, isOutput=True)

    with tile.TileContext(nc) as tc:
        with (
            tc.tile_pool(name="xpool", bufs=1) as xpool,
            tc.tile_pool(name="ypool", bufs=1) as ypool,
            tc.tile_pool(name="psum", bufs=6, space="PSUM") as psum_pool,
            tc.tile_pool(name="stage", bufs=2) as stage_pool,
        ):
            X = xpool.tile([C, PR * PW, PA * PB], F16)
            Y = ypool.tile([C, HB + 6, W + 6], F16)

            # Issue input loads in the order the patch-row pipeline consumes
            # them (the HW queue drains FIFO).  Few, large transfers keep the
            # DMA ring at full bandwidth: x in 2-patch-row chunks (1.05 MB),
            # y in 8-row chunks (1.06 MB).  Patch-row pr needs X chunk pr//2
            # and Y rows [4pr, 4pr+10) = Y chunks pr//2 and pr//2+1.
            def load_x(cx):  # 2 patch-rows of x
                nc.sync.dma_start(
                    X[:, cx * 2 * PW : (cx + 1) * 2 * PW, :],
                    xb[:, cx * 2 * PW : (cx + 1) * 2 * PW, :],
                )

            def load_y(ch):  # Y chunk = 8 rows (last chunk 6 rows)
                r0, r1 = ch * 8, min(ch * 8 + 8, HB + 6)
                nc.sync.dma_start(Y[:, r0:r1, :], yb[:, r0:r1, :])

            load_x(0)
            load_y(0)
            load_y(1)
            for cx in range(1, PR // 2):
                load_x(cx)
                load_y(cx + 1)

            for pr in range(PR):
                # One staging buffer and one output DMA per patch-row keep
                # the Sync sequencer's per-DMA dispatch (~0.6us) off the
                # critical path.
                st = stage_pool.tile([128, QG, NF], F16)
                for gg in range(0, QG, 2):
                    # Eight 4x8 patches (two col-tiled quads) share one PSUM
                    # bank; their evacuation is a single strided copy.
                    ps = psum_pool.tile([128, 2, NF], mybir.dt.float32)
                    for s in range(2):
                        g = gg + s
                        for t in range(G):
                            wp = g * G + t
                            lhsT = X[:, pr * PW + wp, :]
                            rhs = Y[
                                :, pr * PA : pr * PA + HA, wp * PB : wp * PB + HW_
                            ]
                            nc.tensor.matmul(
                                ps[32 * t : 32 * (t + 1), s, :],
                                lhsT,
                                rhs,
                                start=True,
                                stop=True,
                                tile_position=(0, 32 * t),
                            )
                    dst = st[:, gg : gg + 2, :]
                    # Alternate evacuation between DVE and ACT so neither
                    # becomes the bottleneck.
                    if (gg // 2) % 2 == 0:
                        nc.vector.tensor_copy(dst, ps)
                    else:
                        nc.scalar.copy(dst, ps)
                # Dumps ride the Activation engine's HWDGE queue, separate
                # from the input loads on the Sync queue: outputs overlap
                # inputs instead of queueing FIFO behind them (which would
                # stall the stage-buffer recycle until all loads drain).
                nc.scalar.dma_start(corr[pr], st)

    nc.compile()
    return nc


def _program():
    global _PROGRAM
    if _PROGRAM is None:
        _PROGRAM = _build_program()
    return _PROGRAM


def _make_in_maps(x: np.ndarray, y: np.ndarray):
    x0 = np.asarray(x[0]).astype(np.float16)
    # [C, H, W] -> [C, H/PA, PA, PW, PB] -> [C, H/PA, PW, PA, PB]
    xt = x0.reshape(C, H // PA, PA, PW, PB).transpose(0, 1, 3, 2, 4)
    xt = np.ascontiguousarray(xt.reshape(C, H // PA * PW, PA * PB))
    yp = np.zeros((C, H + 6, W + 6), np.float16)
    yp[:, 3 : 3 + H, 3 : 3 + W] = y[0]
    in_maps = []
    for k in range(NCORES):
        in_maps.append(
            {
                "xb": np.ascontiguousarray(xt[:, k * PR * PW : (k + 1) * PR * PW, :]),
                "yb": np.ascontiguousarray(yp[:, k * HB : k * HB + HB + 6, :]),
            }
        )
    return in_maps


_GATHER_IDX = None


def _gather_indices():
    global _GATHER_IDX
    if _GATHER_IDX is None:
        a = np.arange(PA)[:, None, None, None]
        b = np.arange(PB)[None, :, None, None]
        j = np.arange(7)[None, None, :, None]
        i = np.arange(7)[None, None, None, :]
        # n offset for pixel (a, b) and shift (j, i), flattened over (j, i)
        n_idx = ((a + j) * HW_ + (b + i)).reshape(1, 1, PA, PB, 1, 49)
        _GATHER_IDX = np.ascontiguousarray(n_idx)
    return _GATHER_IDX


def _gather_core(corr_k: np.ndarray) -> np.ndarray:
    """[PR, 128, QG, NF] -> [49, HB, W] band of the output."""
    n_idx = _gather_indices()
    ck = corr_k.reshape(PR, G, PA, PB, QG, NF)
    g = np.take_along_axis(ck, n_idx, axis=5)
    # g: [PR, t, a, b, q, 49]; out[s, pr*PA+a, (q*G+t)*PB+b] = g[pr, t, a, b, q, s]
    g = g.transpose(5, 0, 2, 4, 1, 3).reshape(49, HB, W)
    return g


def _run(in_maps, trace=False, **kw):
    return run_bass_kernel_spmd(
        _program(), in_maps, core_ids=list(range(NCORES)), trace=trace, **kw
    )


def kernel(x: np.ndarray, y: np.ndarray) -> np.ndarray:
    x = np.asarray(x)
    y = np.asarray(y)
    res = _run(_make_in_maps(x, y)).results
    out = np.empty((1, 49, H, W), np.float32)
    for k in range(NCORES):
        out[0, :, k * HB : (k + 1) * HB, :] = _gather_core(
            np.asarray(res[k]["corr"])
        ).astype(np.float32)
    return out
